# revision 27
# baseline (speedup 1.0000x reference)
"""Trainium2 Bass kernel for fused additive-attention pooling (nn_Attention).

Reference computes, per batch b:
    logits = enc[b] @ w_enc + (dec[b] @ w_dec + bias)   # second term constant over L
    attn   = softmax(logits)                            # over L
    out[b] = attn @ enc[b]                              # [1, D]

Softmax is shift-invariant, so the decoder/bias term drops out exactly and the
output depends only on encoder_output and w_enc = W[:D, 0].  Per batch the
kernel computes (all fp32, exact):
    s_l  = enc[b,l,:] . w_enc      one fused DVE scalar_tensor_tensor
                                   (elementwise mul + free-axis accumulate)
    p    = exp(s)                  ScalarE (no max-subtraction needed:
                                   s ~ N(0, 0.5), exp is fp32-safe)
    Z    = sum_l p_l               PE matmul accumulation
    out  = (p^T @ enc[b]) / Z      PE fp32 matmuls into PSUM, scaled on ScalarE

Sharding: data-parallel over batch B=32 across 8 NeuronCores (4 batches/core).
Each core streams its 32 MiB enc shard once from HBM; DVE, ScalarE and PE all
run concurrently with the DMA stream.
"""

import sys

if "/opt/trn_rl_repo" not in sys.path:
    sys.path.insert(0, "/opt/trn_rl_repo")

import numpy as np

import concourse.bacc as bacc
import concourse.mybir as mybir
import concourse.tile as tile
from concourse import bass_utils

B, L, D = 32, 2048, 1024
NCORES = 8
B_LOC = B // NCORES          # 4 batches per core
P = 128                      # SBUF partitions
NT = L // P                  # 16 L-tiles of [128, 1024] per batch

TPD = 4                      # L-tiles per dma_start (4 -> 2 MiB transfers)
ENC_BUFS = 3                 # enc tile pool slots (each [128, TPD, 1024])
PROD_BUFS = 4                # product scratch slots


def _build(reps=1):
    nc = bacc.Bacc("TRN2", target_bir_lowering=False, debug=False, num_devices=NCORES)
    f32 = mybir.dt.float32
    enc = nc.dram_tensor("enc", [B_LOC * L, D], f32, kind="ExternalInput")
    wenc = nc.dram_tensor("wenc", [1, D], f32, kind="ExternalInput")
    out = nc.dram_tensor("out", [B_LOC, D], f32, kind="ExternalOutput")

    with tile.TileContext(nc) as tc:
        with (
            tc.tile_pool(name="const", bufs=1) as const_pool,
            tc.tile_pool(name="encp", bufs=ENC_BUFS) as enc_pool,
            tc.tile_pool(name="prod", bufs=PROD_BUFS) as prod_pool,
            tc.tile_pool(name="sp", bufs=4) as s_pool,
            tc.tile_pool(name="pp", bufs=4) as p_pool,
            tc.tile_pool(name="outp", bufs=2) as out_pool,
            tc.tile_pool(name="recip", bufs=2) as recip_pool,
            tc.tile_pool(name="psctx", bufs=2, space="PSUM") as ps_ctx,
            tc.tile_pool(name="psz", bufs=2, space="PSUM") as ps_z,
        ):
            # w_enc broadcast to all 128 partitions, once
            w_row = const_pool.tile([1, D], f32)
            nc.sync.dma_start(w_row[:], wenc[:])
            w_bcast = const_pool.tile([P, D], f32)
            nc.gpsimd.partition_broadcast(w_bcast[:], w_row[:])
            ones = const_pool.tile([P, 1], f32)
            nc.vector.memset(ones[:], 1.0)

            # Cold-start warmups, overlapped with the first DMA fills:
            # fire the ACT exp table load (~2.7us) now instead of on the
            # first real exp, and keep the PE busy so the HAM clock gate
            # reaches full rate before the first real matmul.
            warm = recip_pool.tile([1, 1], f32)
            nc.scalar.activation(
                warm[:], ones[0:1, :], mybir.ActivationFunctionType.Exp
            )
            wps = ps_z.tile([1, 1], f32)
            for i in range(48):
                nc.tensor.matmul(wps[:], ones[:], ones[:])

            for _ in range(reps):
                for b in range(B_LOC):
                    z = ps_z.tile([1, 1], f32)          # sum(p) accumulator
                    ctx = ps_ctx.tile([1, D], f32)      # p^T @ enc accumulator
                    views = [None] * NT
                    for t in range(NT):
                        if t % TPD == 0:
                            r0 = (b * NT + t) * P
                            buf = enc_pool.tile([P, TPD, D], f32)
                            nc.sync.dma_start(
                                buf[:],
                                enc[r0 : r0 + TPD * P, :].rearrange(
                                    "(t p) d -> p t d", p=P
                                ),
                            )
                            for j in range(TPD):
                                views[t + j] = buf[:, j, :]
                        v = views[t]
                        # s[l] = sum_d enc[l,d] * w[d] — one fused DVE op:
                        # prod = enc * w_bcast, accum_out = row-sum(prod)
                        prod = prod_pool.tile([P, D], f32)
                        s = s_pool.tile([P, 1], f32)
                        nc.vector.scalar_tensor_tensor(
                            out=prod[:],
                            in0=v,
                            scalar=1.0,
                            in1=w_bcast[:],
                            op0=mybir.AluOpType.bypass,
                            op1=mybir.AluOpType.mult,
                            accum_out=s[:],
                        )
                        p = p_pool.tile([P, 1], f32)
                        nc.scalar.activation(
                            p[:], s[:], mybir.ActivationFunctionType.Exp
                        )
                        st, sp = t == 0, t == NT - 1
                        nc.tensor.matmul(
                            ctx[:, 0:512], p[:], v[:, 0:512], start=st, stop=sp
                        )
                        nc.tensor.matmul(
                            ctx[:, 512:1024], p[:], v[:, 512:1024], start=st, stop=sp
                        )
                        nc.tensor.matmul(z[:], p[:], ones[:], start=st, stop=sp)
                    recip = recip_pool.tile([1, 1], f32)
                    nc.vector.reciprocal(recip[:], z[:])
                    o = out_pool.tile([1, D], f32)
                    nc.scalar.activation(
                        o[:],
                        ctx[:],
                        mybir.ActivationFunctionType.Copy,
                        scale=recip[:],
                    )
                    nc.sync.dma_start(out[b : b + 1, :], o[:])
    nc.compile()
    return nc


_NC = None


def _get_nc():
    global _NC
    if _NC is None:
        _NC = _build()
    return _NC


def _run(nc, enc_np, wenc_np, **kwargs):
    in_maps = [
        {
            "enc": np.ascontiguousarray(
                enc_np[i * B_LOC : (i + 1) * B_LOC].reshape(B_LOC * L, D)
            ),
            "wenc": wenc_np,
        }
        for i in range(NCORES)
    ]
    res = bass_utils.run_bass_kernel_spmd(
        nc, in_maps, core_ids=list(range(NCORES)), **kwargs
    )
    ctxs = np.concatenate([r["out"] for r in res.results], axis=0)  # [B, D]
    return ctxs.reshape(B, 1, D).astype(np.float32), res


def kernel(encoder_output, decoder_hidden=None, W=None, b=None):
    enc_np = np.asarray(encoder_output, dtype=np.float32)
    wenc_np = np.ascontiguousarray(np.asarray(W, dtype=np.float32)[:D, 0]).reshape(1, D)
    out, _ = _run(_get_nc(), enc_np, wenc_np)
    return out


# revision 29
# speedup vs baseline: 1.0450x; 1.0450x over previous
"""Trainium2 Bass kernel for fused additive-attention pooling (nn_Attention).

Reference computes, per batch b:
    logits = enc[b] @ w_enc + (dec[b] @ w_dec + bias)   # second term constant over L
    attn   = softmax(logits)                            # over L
    out[b] = attn @ enc[b]                              # [1, D]

Softmax is shift-invariant, so the decoder/bias term drops out exactly and the
output depends only on encoder_output and w_enc = W[:D, 0].  Per batch the
kernel computes (all fp32, exact):
    s_l  = enc[b,l,:] . w_enc      one fused DVE scalar_tensor_tensor
                                   (elementwise mul + free-axis accumulate)
    p    = exp(s)                  ScalarE (no max-subtraction needed:
                                   s ~ N(0, 0.5), exp is fp32-safe)
    Z    = sum_l p_l               PE matmul accumulation
    out  = (p^T @ enc[b]) / Z      PE fp32 matmuls into PSUM, scaled on ScalarE

Sharding: data-parallel over batch B=32 across 8 NeuronCores (4 batches/core).
Each core streams its 32 MiB enc shard once from HBM; DVE, ScalarE and PE all
run concurrently with the DMA stream.
"""

import sys

if "/opt/trn_rl_repo" not in sys.path:
    sys.path.insert(0, "/opt/trn_rl_repo")

import numpy as np

import concourse.bacc as bacc
import concourse.mybir as mybir
import concourse.tile as tile
from concourse import bass_utils

B, L, D = 32, 2048, 1024
NCORES = 8
B_LOC = B // NCORES          # 4 batches per core
P = 128                      # SBUF partitions
NT = L // P                  # 16 L-tiles of [128, 1024] per batch

TPD = 1                      # L-tiles per dma_start (1 -> 512 KiB transfers)
ENC_BUFS = 8                 # enc tile pool slots (each [128, TPD, 1024])
PROD_BUFS = 4                # product scratch slots


def _build(reps=1):
    nc = bacc.Bacc("TRN2", target_bir_lowering=False, debug=False, num_devices=NCORES)
    f32 = mybir.dt.float32
    enc = nc.dram_tensor("enc", [B_LOC * L, D], f32, kind="ExternalInput")
    wenc = nc.dram_tensor("wenc", [1, D], f32, kind="ExternalInput")
    out = nc.dram_tensor("out", [B_LOC, D], f32, kind="ExternalOutput")

    with tile.TileContext(nc) as tc:
        with (
            tc.tile_pool(name="const", bufs=1) as const_pool,
            tc.tile_pool(name="encp", bufs=ENC_BUFS) as enc_pool,
            tc.tile_pool(name="prod", bufs=PROD_BUFS) as prod_pool,
            tc.tile_pool(name="sp", bufs=4) as s_pool,
            tc.tile_pool(name="pp", bufs=4) as p_pool,
            tc.tile_pool(name="outp", bufs=2) as out_pool,
            tc.tile_pool(name="recip", bufs=2) as recip_pool,
            tc.tile_pool(name="psctx", bufs=2, space="PSUM") as ps_ctx,
            tc.tile_pool(name="psz", bufs=2, space="PSUM") as ps_z,
        ):
            # w_enc broadcast to all 128 partitions, once
            w_row = const_pool.tile([1, D], f32)
            nc.sync.dma_start(w_row[:], wenc[:])
            w_bcast = const_pool.tile([P, D], f32)
            nc.gpsimd.partition_broadcast(w_bcast[:], w_row[:])
            ones = const_pool.tile([P, 1], f32)
            nc.vector.memset(ones[:], 1.0)

            # Cold-start warmups, overlapped with the first DMA fills:
            # fire the ACT exp table load (~2.7us) now instead of on the
            # first real exp, and keep the PE busy so the HAM clock gate
            # reaches full rate before the first real matmul.
            warm = recip_pool.tile([1, 1], f32)
            nc.scalar.activation(
                warm[:], ones[0:1, :], mybir.ActivationFunctionType.Exp
            )
            wps = ps_z.tile([1, 1], f32)
            for i in range(48):
                nc.tensor.matmul(wps[:], ones[:], ones[:])

            for _ in range(reps):
                for b in range(B_LOC):
                    z = ps_z.tile([1, 1], f32)          # sum(p) accumulator
                    ctx = ps_ctx.tile([1, D], f32)      # p^T @ enc accumulator
                    views = [None] * NT
                    for t in range(NT):
                        if t % TPD == 0:
                            r0 = (b * NT + t) * P
                            buf = enc_pool.tile([P, TPD, D], f32)
                            nc.sync.dma_start(
                                buf[:],
                                enc[r0 : r0 + TPD * P, :].rearrange(
                                    "(t p) d -> p t d", p=P
                                ),
                            )
                            for j in range(TPD):
                                views[t + j] = buf[:, j, :]
                        v = views[t]
                        # s[l] = sum_d enc[l,d] * w[d] — one fused DVE op:
                        # prod = enc * w_bcast, accum_out = row-sum(prod)
                        prod = prod_pool.tile([P, D], f32)
                        s = s_pool.tile([P, 1], f32)
                        nc.vector.scalar_tensor_tensor(
                            out=prod[:],
                            in0=v,
                            scalar=1.0,
                            in1=w_bcast[:],
                            op0=mybir.AluOpType.bypass,
                            op1=mybir.AluOpType.mult,
                            accum_out=s[:],
                        )
                        p = p_pool.tile([P, 1], f32)
                        nc.scalar.activation(
                            p[:], s[:], mybir.ActivationFunctionType.Exp
                        )
                        st, sp = t == 0, t == NT - 1
                        nc.tensor.matmul(
                            ctx[:, 0:512], p[:], v[:, 0:512], start=st, stop=sp
                        )
                        nc.tensor.matmul(
                            ctx[:, 512:1024], p[:], v[:, 512:1024], start=st, stop=sp
                        )
                        nc.tensor.matmul(z[:], p[:], ones[:], start=st, stop=sp)
                    recip = recip_pool.tile([1, 1], f32)
                    nc.vector.reciprocal(recip[:], z[:])
                    o = out_pool.tile([1, D], f32)
                    nc.scalar.activation(
                        o[:],
                        ctx[:],
                        mybir.ActivationFunctionType.Copy,
                        scale=recip[:],
                    )
                    nc.sync.dma_start(out[b : b + 1, :], o[:])
    nc.compile()
    return nc


_NC = None


def _get_nc():
    global _NC
    if _NC is None:
        _NC = _build()
    return _NC


def _run(nc, enc_np, wenc_np, **kwargs):
    in_maps = [
        {
            "enc": np.ascontiguousarray(
                enc_np[i * B_LOC : (i + 1) * B_LOC].reshape(B_LOC * L, D)
            ),
            "wenc": wenc_np,
        }
        for i in range(NCORES)
    ]
    res = bass_utils.run_bass_kernel_spmd(
        nc, in_maps, core_ids=list(range(NCORES)), **kwargs
    )
    ctxs = np.concatenate([r["out"] for r in res.results], axis=0)  # [B, D]
    return ctxs.reshape(B, 1, D).astype(np.float32), res


def kernel(encoder_output, decoder_hidden=None, W=None, b=None):
    enc_np = np.asarray(encoder_output, dtype=np.float32)
    wenc_np = np.ascontiguousarray(np.asarray(W, dtype=np.float32)[:D, 0]).reshape(1, D)
    out, _ = _run(_get_nc(), enc_np, wenc_np)
    return out
